# revision 55
# baseline (speedup 1.0000x reference)
"""GCN forward (4-layer GCNConv + global mean-pool + linear) on 8 TRN2 cores.

Strategy (graph/dst-node data parallelism per the sharding hint):
  * Associativity: S @ (h W) == (S @ h) W  -> message passing at *input* width.
  * Symmetric norm factored: agg_d = dinv_d * (sum_{s->d} dinv_s h_s + dinv_d h_d);
    the self-loop term is applied algebraically (one DVE add per tile), keeping
    self-edges out of the gather schedule entirely.
  * Nodes relabeled + serpentine-packed (equal degree sum) into T tiles of 128
    dst slots per core; core c owns the edges pointing at its tiles.  The edge
    schedule is static and shared across cores: per (tile, src-chunk) the slot
    count is 128 * ceil(max_over_cores(count)/128); per-core tables pad with
    slot=-1 (matches nothing).
  * Per layer: dma_gather (256B bf16 rows) fetches source features per edge;
    DVE builds a one-hot matrix A = (slot_id == iota) per 128-edge group and
    TensorE computes psum[dst_slot, :] += A^T @ G  (the segment sum).
  * agg -> (dinv_d scale) -> PE transpose -> W matmul -> bias + PReLU ->
    PE transpose back -> dinv scale -> store -> AllGather (layer input h too).
  * Mean-pool via on-chip one-hot (graph-id == iota) matmuls; 1/cnt and the
    AllReduce + final linear applied on the [64, 4] tail.

Host preprocessing (degrees, packing, index tables) is vectorized numpy.
Inputs are packed into 4 arrays per core (~1 MB each core, ~8 MB total).
"""

import os

import jax

os.makedirs("/tmp/jax_cache", exist_ok=True)
jax.config.update("jax_compilation_cache_dir", "/tmp/jax_cache")
jax.config.update("jax_persistent_cache_min_entry_size_bytes", 0)
jax.config.update("jax_persistent_cache_min_compile_time_secs", 0)

import numpy as np
import ml_dtypes

import concourse.bacc as bacc
import concourse.mybir as mybir
import concourse.tile as tile
from concourse.bass import ts
from concourse.bass_utils import run_bass_kernel_spmd
from concourse.library_config import mlp as mlp_lib
from concourse.masks import make_identity

F32 = mybir.dt.float32
BF16 = mybir.dt.bfloat16
I16 = mybir.dt.int16
I8 = mybir.dt.int8

GW = 128             # gather row width in bf16 (= 256B, dma_gather minimum)
MAX_CHUNK = 32768    # int16 gather-index limit over the node space
_ABLATE = None       # timing experiments only: "nogather" / "nogmm"
GBLK = 8             # groups (of 128 idx) per dma_gather instruction
NQ = 4               # SWDGE queues
_G_SPACE = "Shared"  # addr_space for AllGather output tensors
_BLOB = False        # ship all inputs as one packed int8 array per core


# ------------------------------------------------------------------ host prep
def _preprocess(x, edge_src, edge_dst, batch, n_cores, num_graphs):
    N, IN_FEAT = x.shape
    src = edge_src.astype(np.int64)
    dst = edge_dst.astype(np.int64)
    deg = np.bincount(dst, minlength=N).astype(np.int64) + 1   # + self-loop
    dinv = (1.0 / np.sqrt(deg)).astype(np.float32)

    core_of = np.arange(N) % n_cores
    per_core = np.bincount(core_of, minlength=n_cores)
    T = int(-(-per_core.max() // 128))
    T += T % 2
    Npad = n_cores * T * 128

    # serpentine deal of degree-sorted nodes -> near-equal degree per tile
    tile_of = np.empty(N, np.int64)
    slot_of = np.empty(N, np.int64)
    for c in range(n_cores):
        nodes_c = np.where(core_of == c)[0]
        order = nodes_c[np.argsort(-deg[nodes_c], kind="stable")]
        pos = np.arange(len(order))
        rnd, k = pos // T, pos % T
        tile_of[order] = np.where(rnd % 2 == 0, k, T - 1 - k)
        slot_of[order] = rnd
    assert slot_of.max() < 128

    gid = core_of * (T * 128) + tile_of * 128 + slot_of
    sg = gid[src]
    core_e = core_of[dst]
    tile_d = tile_of[dst]
    slot_d = slot_of[dst].astype(np.int8)

    nchunk = int(-(-Npad // MAX_CHUNK))
    chunk_rows = Npad // nchunk
    assert chunk_rows <= MAX_CHUNK

    chunk_e = sg // chunk_rows
    srcloc_e = (sg % chunk_rows).astype(np.int16)

    # per (core, tile, chunk) counts -> shared schedule from max over cores
    key = (core_e * T + tile_d) * nchunk + chunk_e
    cnt = np.bincount(key, minlength=n_cores * T * nchunk)
    cnt = cnt.reshape(n_cores, T, nchunk)
    gpc = -(-cnt.max(axis=0) // 128)          # [T, nchunk] groups per (t, ch)
    last_ch = nchunk - 1 - np.argmax(gpc[:, ::-1] > 0, axis=1)

    # 32-tile batches, alternating 4-bank psum sets -> batch n+1 aggregation
    # overlaps batch n psum drain
    batches = [list(range(b, min(b + 32, T))) for b in range(0, T, 32)]
    base_tc = np.full((T, nchunk), -1, np.int64)
    sched = []   # per (b, ch): (base, npos, [(w, t, ngroups)])
    pos = 0
    for btiles in batches:
        for ch in range(nchunk):
            base = pos
            tl_list = []
            for w, t in enumerate(btiles):
                g = int(gpc[t, ch])
                if g == 0:
                    continue
                base_tc[t, ch] = pos
                tl_list.append((w, t, g))
                pos += g * 128
            sched.append((base, pos - base, tl_list))
    S = pos
    assert S % 128 == 0

    # group metadata: (w, stop) in position order, per (b, ch) entry
    groups_per_sched = []
    si = 0
    for btiles in batches:
        for ch in range(nchunk):
            base, npos, tl_list = sched[si]
            si += 1
            glist = []
            for w, t, g in tl_list:
                for j in range(g):
                    sp = (ch == last_ch[t]) and (j == g - 1)
                    glist.append((w, sp))
            groups_per_sched.append(glist)

    # per-core edge tables packed into the shared schedule
    idx_tbl = np.zeros((n_cores, 16, S // 16), dtype=np.int16)
    s_tblf = np.full((n_cores, S), -1, dtype=np.int8)
    base_flat = base_tc.reshape(-1)
    for c in range(n_cores):
        m = core_e == c
        k2 = tile_d[m] * nchunk + chunk_e[m]
        # sort by src within each run: monotone gather addresses (HBM locality)
        order = np.lexsort((srcloc_e[m], k2))
        k2s = k2[order]
        starts = np.searchsorted(k2s, np.arange(T * nchunk))
        within = np.arange(len(k2s)) - starts[k2s]
        pos_of = base_flat[k2s] + within
        idx_flat = np.zeros(S, np.int16)
        idx_flat[pos_of] = srcloc_e[m][order]
        s_tblf[c][pos_of] = slot_d[m][order]
        idx_tbl[c] = idx_flat.reshape(S // 16, 16).T

    # per-node tables in (core, slot_p, tile) layout
    lin = core_of * (128 * T) + slot_of * T + tile_of
    xd = np.zeros((n_cores * 128 * T, IN_FEAT), np.float32)
    xd[lin] = x * dinv[:, None]
    xd = xd.reshape(n_cores, 128, T * IN_FEAT)
    b_of = np.full(n_cores * 128 * T, -1, np.int8)
    b_of[lin] = batch.astype(np.int8)
    b_of = b_of.reshape(n_cores, 128, T)
    dinv_my = np.ones(n_cores * 128 * T, np.float32)
    dinv_my[lin] = dinv
    dinv_my = dinv_my.reshape(n_cores, 128, T)

    cntg = np.bincount(batch, minlength=num_graphs).astype(np.float32)
    invc = (1.0 / np.maximum(cntg, 1.0)).astype(np.float32)

    # packed int8 tables: [128, S//128 (slots) + T (graph ids)]
    si8 = np.concatenate(
        [s_tblf.reshape(n_cores, S // 128, 128).transpose(0, 2, 1), b_of], axis=2)

    return dict(T=T, S=S, Npad=Npad, chunk_rows=chunk_rows, nchunk=nchunk,
                batches=batches, sched=sched, groups_per_sched=groups_per_sched,
                idx_tbl=idx_tbl, si8=si8, xd=xd, dinv_my=dinv_my, invc=invc)


def _pack_inputs(meta, n_cores, IN_FEAT, widths, out_widths, num_graphs,
                 n_classes, Ws, bs, Wlin, blin):
    """Pack weights + per-node tables into one bf16 and one f32 array/core."""
    T = meta["T"]
    NL = len(widths)
    wcols = int(np.sum(out_widths))
    # bf16 pack: [128, wcols + T*IN_FEAT] = W1..W4 | xd
    pbf = np.zeros((n_cores, 128, wcols + T * IN_FEAT), np.float32)
    c0 = 0
    for i in range(NL):
        pbf[:, :widths[i], c0:c0 + out_widths[i]] = np.asarray(Ws[i], np.float32)
        c0 += out_widths[i]
    pbf[:, :, c0:] = meta["xd"]
    pbf = pbf.astype(ml_dtypes.bfloat16)
    # f32 pack: [128, T + NL + NL + n_classes + n_classes + 1]
    #   dinv_my | b_i | bn_i | Wlin | blin | invc
    fcols = T + 2 * NL + 2 * n_classes + 1
    pf = np.zeros((n_cores, 128, fcols), np.float32)
    pf[:, :, :T] = meta["dinv_my"]
    for i in range(NL):
        pf[:, :out_widths[i], T + i] = np.asarray(bs[i], np.float32)
        pf[:, :out_widths[i], T + NL + i] = -np.asarray(bs[i], np.float32)
    Wl = np.asarray(Wlin, np.float32)
    pf[:, :Wl.shape[0], T + 2 * NL:T + 2 * NL + n_classes] = Wl
    pf[:, :num_graphs, T + 2 * NL + n_classes:T + 2 * NL + 2 * n_classes] = \
        np.asarray(blin, np.float32)[None, None, :]
    pf[:, :num_graphs, -1] = meta["invc"]
    return pbf, pf


# ------------------------------------------------------------------ device IR
def _build(meta, n_cores, IN_FEAT, widths, out_widths, num_graphs, n_classes,
           alphas):
    T, S, Npad = meta["T"], meta["S"], meta["Npad"]
    chunk_rows, nchunk = meta["chunk_rows"], meta["nchunk"]
    batches, sched = meta["batches"], meta["groups_per_sched"]
    sched_pos = meta["sched"]
    NL = len(widths)
    nodes_my = T * 128
    wcols = int(np.sum(out_widths))
    bfcols = wcols + T * IN_FEAT
    fcols = T + 2 * NL + 2 * n_classes + 1
    woff = np.cumsum([0] + list(out_widths))[:-1]
    OX, OD, OB, OBN = wcols, 0, T, T + NL
    OWL, OBL, OIC = T + 2 * NL, T + 2 * NL + n_classes, fcols - 1

    nc = bacc.Bacc("TRN2", target_bir_lowering=False, debug=False,
                   num_devices=n_cores, num_swdge_queues=NQ)
    rg = [list(range(n_cores))]

    sicols = S // 128 + T
    if _BLOB:
        # byte layout: idx (2S) | si8 (128*sicols) | pbf (2*128*bfcols)
        #              | pf (4*128*fcols)
        NB = 2 * S + 128 * sicols + 2 * 128 * bfcols + 4 * 128 * fcols
        blob_in = nc.dram_tensor("blob", [NB, 1], I8, kind="ExternalInput")
        A1 = 2 * S
        A2 = A1 + 128 * sicols
        A3 = A2 + 2 * 128 * bfcols
    else:
        pbf_in = nc.dram_tensor("pbf", [128, bfcols], BF16, kind="ExternalInput")
        pf_in = nc.dram_tensor("pf", [128, fcols], F32, kind="ExternalInput")
        si8_in = nc.dram_tensor("si8", [128, sicols], I8, kind="ExternalInput")
        idx_in = nc.dram_tensor("idx_tbl", [16, S // 16], I16,
                                kind="ExternalInput")
    out_t = nc.dram_tensor("out", [num_graphs, n_classes], F32,
                           kind="ExternalOutput")

    g = [nc.dram_tensor(f"g{i+1}", [Npad, GW], BF16, addr_space=_G_SPACE)
         for i in range(NL)]
    h_slice = [nc.dram_tensor(f"hs{i+1}", [nodes_my, GW], BF16)
               for i in range(NL)]
    pooled_d = nc.dram_tensor("pooled", [128, num_graphs], F32)
    pooled_r = nc.dram_tensor("pooled_red", [128, num_graphs], F32)

    with tile.TileContext(nc) as tc:
        with (
            tc.tile_pool(name="const", bufs=1) as cpool,
            tc.tile_pool(name="meta", bufs=2) as mpool,
            tc.tile_pool(name="gat", bufs=8) as gpool,
            tc.tile_pool(name="am", bufs=8) as apool,
            tc.tile_pool(name="big", bufs=1) as bpool,
            tc.tile_pool(name="ps", bufs=1, space="PSUM") as pspool,
        ):
            nc.gpsimd.load_library(mlp_lib)

            iden = cpool.tile([128, 128], BF16)
            make_identity(nc, iden[:])
            iota = cpool.tile([128, 128], BF16)
            nc.gpsimd.iota(iota[:], [[1, 128]], channel_multiplier=0,
                           allow_small_or_imprecise_dtypes=True)

            pbf = cpool.tile([128, bfcols], BF16, tag="pbf")
            pf = cpool.tile([128, fcols], F32, tag="pf")
            si8 = cpool.tile([128, sicols], I8, tag="si8")
            idx_all = cpool.tile([128, S // 16], I16, tag="idx_all")
            if _BLOB:
                def bview(a, b, p):
                    return blob_in.ap()[a:b, :].rearrange(
                        "(p c) o -> p (c o)", p=p)
                nc.sync.dma_start(pbf[:].bitcast(I8), bview(A2, A3, 128))
                nc.sync.dma_start(pf[:].bitcast(I8), bview(A3, NB, 128))
                nc.sync.dma_start(si8[:], bview(A1, A2, 128))
                idx_src = bview(0, 2 * S, 16)
                for k in range(8):
                    nc.sync.dma_start(idx_all[k * 16:(k + 1) * 16, :].bitcast(I8),
                                      idx_src)
            else:
                nc.sync.dma_start(pbf[:], pbf_in.ap())
                nc.sync.dma_start(pf[:], pf_in.ap())
                nc.sync.dma_start(si8[:], si8_in.ap())
                for k in range(8):
                    nc.sync.dma_start(idx_all[k * 16:(k + 1) * 16, :],
                                      idx_in.ap())
            sb16 = cpool.tile([128, sicols], BF16, tag="sb16")
            nc.vector.tensor_copy(sb16[:], si8[:])
            s_all = sb16[:, :S // 128]
            b_of16 = sb16[:, S // 128:]
            dinv_my = pf[:, OD:OD + T]

            aggT = bpool.tile([128, nodes_my], BF16, tag="aggT")
            h_sb = bpool.tile([128, nodes_my], BF16, tag="h_sb")
            gnext = bpool.tile([128, T * GW], BF16, tag="gnext")
            agg = bpool.tile([128, T * 64], BF16, tag="agg")

            # ---------------- layer-1 input: my slice of dinv*x -> AllGather
            nc.vector.tensor_copy(
                gnext[:].rearrange("p (t f) -> p t f", f=GW)[:, :, :IN_FEAT],
                pbf[:, OX:].rearrange("p (t f) -> p t f", f=IN_FEAT))
            hs0v = h_slice[0].ap().rearrange("(t p) f -> p t f", p=128)
            nc.sync.dma_start(hs0v[:], gnext[:].rearrange("p (t f) -> p t f", f=GW))
            if n_cores > 1:
                nc.gpsimd.collective_compute(
                    "AllGather", mybir.AluOpType.bypass, rg,
                    [h_slice[0].ap()], [g[0].ap()])
            else:
                nc.sync.dma_start(g[0].ap()[:nodes_my, :], h_slice[0].ap())

            gq_counter = [0]
            pooling_psum = None

            for li in range(NL):
                F, Fo = widths[li], out_widths[li]
                gsrc = g[li]
                # ---- aggregation
                for bi, btiles in enumerate(batches):
                    b0 = (bi % 2) * 4
                    psum = []
                    for k in range(4):
                        pst = pspool.tile([128, 512], F32, tag=f"ps{b0 + k}",
                                          name=f"pst{b0 + k}")
                        nc.vector.memset(pst[:], 0.0)
                        psum.append(pst)
                    for ch in range(nchunk):
                        base, npos, _tl = sched_pos[bi * nchunk + ch]
                        glist = sched[bi * nchunk + ch]
                        crows = min(chunk_rows, Npad - ch * chunk_rows)
                        srcv = gsrc.ap()[ch * chunk_rows:ch * chunk_rows + crows, :]
                        ngroups = len(glist)
                        for g0 in range(0, ngroups, GBLK):
                            ng = min(GBLK, ngroups - g0)
                            nidx = ng * 128
                            gtile = gpool.tile([128, GBLK, GW], BF16, tag="gtile")
                            c0 = base // 16 + g0 * 8
                            if _ABLATE not in ("nogather", "nogmm"):
                                nc.gpsimd.dma_gather(
                                    gtile[:, :ng, :], srcv,
                                    idx_all[:, c0:c0 + nidx // 16],
                                    nidx, nidx, GW,
                                    queue_num=gq_counter[0] % NQ)
                            gq_counter[0] += 1
                            A = apool.tile([128, GBLK, 128], BF16, tag="A")
                            ss = s_all[:, base // 128 + g0:base // 128 + g0 + ng]
                            nc.vector.tensor_tensor(
                                A[:, :ng, :],
                                ss[:, :, None].broadcast_to([128, ng, 128]),
                                iota[:, None, :].broadcast_to([128, ng, 128]),
                                op=mybir.AluOpType.is_equal)
                            for gg in range(ng):
                                w, sp = glist[g0 + gg]
                                if _ABLATE == "nogmm":
                                    continue
                                nc.tensor.matmul(
                                    psum[w % 4][:, (w // 4) * 64:(w // 4) * 64 + F],
                                    A[:, gg, :], gtile[:, gg, :F],
                                    start=False, stop=sp, skip_group_check=True)
                    for w, tl in enumerate(btiles):
                        reg = psum[w % 4][:, (w // 4) * 64:(w // 4) * 64 + F]
                        # self-loop term: agg_d += g_d (pre-dinv scale)
                        nc.vector.tensor_tensor(
                            reg, reg, gnext[:, tl * GW:tl * GW + F],
                            op=mybir.AluOpType.add)
                        nc.scalar.activation(
                            agg[:, tl * 64:tl * 64 + F], reg,
                            mybir.ActivationFunctionType.Identity,
                            scale=dinv_my[:, tl:tl + 1])

                # ---- transpose agg -> aggT [F, nodes]
                for tl in range(T):
                    tp = pspool.tile([128, 512], BF16, tag=f"ps{tl % 2}")
                    nc.tensor.matmul(tp[:F, :128], agg[:, tl * 64:tl * 64 + F],
                                     iden[:], is_transpose=True,
                                     skip_group_check=True)
                    nc.scalar.copy(aggT[:F, tl * 128:(tl + 1) * 128], tp[:F, :128])

                # ---- h^T = W^T @ aggT + bias, PReLU
                a_f = alphas[li] if li < NL - 1 else None
                Wv = pbf[:, woff[li]:woff[li] + Fo]
                bv = pf[:, OB + li:OB + li + 1]
                bnv = pf[:, OBN + li:OBN + li + 1]
                for n0 in range(0, nodes_my, 512):
                    nch = min(512, nodes_my - n0)
                    hp = pspool.tile([128, 512], F32, tag=f"ps{2 + (n0 // 512) % 2}")
                    nc.tensor.matmul(hp[:Fo, :nch], Wv[:F, :],
                                     aggT[:F, n0:n0 + nch], skip_group_check=True)
                    if li < NL - 1:
                        # prelu(x+b) = relu(x+b) - a * relu(-x-b)
                        nc.scalar.activation(
                            h_sb[:Fo, n0:n0 + nch], hp[:Fo, :nch],
                            mybir.ActivationFunctionType.Relu,
                            bias=bv[:Fo, :], scale=1.0)
                        hrelu = mpool.tile([128, 512], BF16, tag="hrelu")
                        nc.scalar.activation(
                            hrelu[:Fo, :nch], hp[:Fo, :nch],
                            mybir.ActivationFunctionType.Relu,
                            bias=bnv[:Fo, :], scale=-1.0)
                        nc.vector.scalar_tensor_tensor(
                            h_sb[:Fo, n0:n0 + nch], hrelu[:Fo, :nch],
                            float(-a_f), h_sb[:Fo, n0:n0 + nch],
                            op0=mybir.AluOpType.mult, op1=mybir.AluOpType.add)
                    else:
                        nc.scalar.activation(
                            h_sb[:Fo, n0:n0 + nch], hp[:Fo, :nch],
                            mybir.ActivationFunctionType.Identity,
                            bias=bv[:Fo, :], scale=1.0)

                # ---- transpose back; dinv-scale (layers 1-3) or pooling (L4)
                for tl in range(T):
                    tp2 = pspool.tile([128, 512], BF16, tag=f"ps{4 + tl % 2}")
                    nc.tensor.matmul(tp2[:128, :Fo],
                                     h_sb[:Fo, tl * 128:(tl + 1) * 128],
                                     iden[:Fo, :Fo], is_transpose=True,
                                     skip_group_check=True)
                    if li < NL - 1:
                        nc.scalar.activation(
                            gnext[:, tl * GW:tl * GW + Fo], tp2[:, :Fo],
                            mybir.ActivationFunctionType.Identity,
                            scale=dinv_my[:, tl:tl + 1])
                    else:
                        h4n = mpool.tile([128, 128], BF16, tag="h4n")
                        nc.vector.tensor_copy(h4n[:, :Fo], tp2[:, :Fo])
                        Ap = apool.tile([128, 64], BF16, tag="Ap")
                        nc.vector.tensor_tensor(
                            Ap[:, :num_graphs],
                            b_of16[:, tl:tl + 1].broadcast_to([128, num_graphs]),
                            iota[:, :num_graphs],
                            op=mybir.AluOpType.is_equal)
                        if pooling_psum is None:
                            pooling_psum = pspool.tile([128, 512], F32, tag="ps6")
                        nc.tensor.matmul(
                            pooling_psum[:Fo, :num_graphs], h4n[:, :Fo],
                            Ap[:, :num_graphs], start=(tl == 0),
                            stop=(tl == T - 1), skip_group_check=True)

                if li < NL - 1:
                    hsv = h_slice[li + 1].ap().rearrange("(t p) f -> p t f", p=128)
                    nc.sync.dma_start(
                        hsv[:], gnext[:].rearrange("p (t f) -> p t f", f=GW))
                    if n_cores > 1:
                        nc.gpsimd.collective_compute(
                            "AllGather", mybir.AluOpType.bypass, rg,
                            [h_slice[li + 1].ap()], [g[li + 1].ap()])
                    else:
                        nc.sync.dma_start(g[li + 1].ap()[:nodes_my, :],
                                          h_slice[li + 1].ap())

            # ---------------- pooled -> AllReduce -> final linear
            Fo = out_widths[-1]
            pooled_sb = cpool.tile([128, num_graphs], F32, tag="pooled")
            nc.vector.tensor_copy(pooled_sb[:Fo, :], pooling_psum[:Fo, :num_graphs])
            if n_cores > 1:
                nc.sync.dma_start(pooled_d.ap()[:Fo, :], pooled_sb[:Fo, :])
                nc.gpsimd.collective_compute(
                    "AllReduce", mybir.AluOpType.add, rg,
                    [pooled_d.ap()], [pooled_r.ap()])
                pooled2 = cpool.tile([128, num_graphs], F32, tag="pooled2")
                nc.sync.dma_start(pooled2[:Fo, :], pooled_r.ap()[:Fo, :])
            else:
                pooled2 = pooled_sb
            fin = pspool.tile([128, 512], F32, tag="ps7")
            nc.tensor.matmul(fin[:num_graphs, :n_classes], pooled2[:Fo, :num_graphs],
                             pf[:Fo, OWL:OWL + n_classes], skip_group_check=True)
            out_sb = cpool.tile([num_graphs, n_classes], F32, tag="outsb")
            nc.vector.tensor_tensor(
                out_sb[:], fin[:num_graphs, :n_classes],
                pf[:num_graphs, OIC:OIC + 1].broadcast_to([num_graphs, n_classes]),
                op=mybir.AluOpType.mult)
            nc.vector.tensor_tensor(out_sb[:], out_sb[:],
                                    pf[:num_graphs, OBL:OBL + n_classes],
                                    op=mybir.AluOpType.add)
            nc.sync.dma_start(out_t.ap(), out_sb[:])

    nc.compile()
    # the jax custom-call lowering re-serializes the (immutable, compiled) BIR
    # on every launch; memoize the bytes once.
    bir_bytes = nc.to_json_bytes()
    nc.to_json_bytes = lambda: bir_bytes
    return nc


def _make_in_maps(meta, n_cores, Ws, bs, Wlin, blin, num_graphs):
    IN_FEAT = 8
    widths = [8, 16, 32, 64]
    out_widths = [16, 32, 64, 128]
    n_classes = np.asarray(Wlin).shape[1]
    pbf, pf = _pack_inputs(meta, n_cores, IN_FEAT, widths, out_widths,
                           num_graphs, n_classes, Ws, bs, Wlin, blin)
    in_maps = []
    for c in range(n_cores):
        if _BLOB:
            blob = np.concatenate([
                np.ascontiguousarray(meta["idx_tbl"][c]).view(np.int8).ravel(),
                np.ascontiguousarray(meta["si8"][c]).ravel(),
                np.ascontiguousarray(pbf[c]).view(np.int8).ravel(),
                np.ascontiguousarray(pf[c]).view(np.int8).ravel(),
            ])
            in_maps.append(dict(blob=blob[:, None]))
        else:
            in_maps.append(dict(
                pbf=np.ascontiguousarray(pbf[c]),
                pf=np.ascontiguousarray(pf[c]),
                si8=np.ascontiguousarray(meta["si8"][c]),
                idx_tbl=np.ascontiguousarray(meta["idx_tbl"][c]),
            ))
    return in_maps


# ------------------------------------------------------------------ entry
def kernel(x, edge_src, edge_dst, batch,
           W1, b1, W2, b2, W3, b3, W4, b4,
           a1, a2, a3, Wlin, blin, n_cores=8):
    x = np.asarray(x, dtype=np.float32)
    edge_src = np.asarray(edge_src, dtype=np.int32)
    edge_dst = np.asarray(edge_dst, dtype=np.int32)
    batch = np.asarray(batch, dtype=np.int32)
    Ws = [np.asarray(w, np.float32) for w in (W1, W2, W3, W4)]
    bs = [np.asarray(b, np.float32) for b in (b1, b2, b3, b4)]
    alphas = [float(a1), float(a2), float(a3)]
    Wlin = np.asarray(Wlin, np.float32)
    blin = np.asarray(blin, np.float32)

    IN_FEAT = x.shape[1]
    widths = [IN_FEAT] + [w.shape[1] for w in Ws[:-1]]
    out_widths = [w.shape[1] for w in Ws]
    NG = 64
    NCLS = Wlin.shape[1]

    meta = _preprocess(x, edge_src, edge_dst, batch, n_cores, NG)
    nc = _build(meta, n_cores, IN_FEAT, widths, out_widths, NG, NCLS, alphas)
    in_maps = _make_in_maps(meta, n_cores, Ws, bs, Wlin, blin, NG)

    res = run_bass_kernel_spmd(nc, in_maps, core_ids=list(range(n_cores)))
    return np.asarray(res.results[0]["out"], dtype=np.float32)
